# revision 1
# baseline (speedup 1.0000x reference)
"""Trainium2 Bass kernel for nn_ExampleNet (2x NNConv edge-conditioned conv
+ global_add_pool + MLP head), distributed over 8 NeuronCores.

Strategy (edge-parallel, dst-sharded):
  - Host sorts edges by dst node; core k owns dst nodes [6250k, 6250(k+1)).
  - Per core, edges are grouped into 128-node scatter windows; segment-sum is
    done ON-CHIP: per 128-edge tile a one-hot matrix A[e,n]=(dst==n) is built
    with one DVE is_equal op, and a PE matmul with lhsT=A, rhs=tmp accumulates
    the UNREDUCED per-edge products into a PSUM window (node-major), fusing
    the message contraction's i-reduction into a single per-window DVE reduce.
    No indirect scatter, no AllReduce.
  - Per-edge MLP: the big second layer (h @ w2, 97% of FLOPs) runs on PE per
    tile; the tiny first layer h=relu(ea@w1+b1) is host-precomputed edge
    feature prep.  x[src] is host-pre-gathered for conv1.
  - Bias, edge-MLP output bias (b2), and the root term are all folded into
    one per-window root matmul via host-augmented features [x | 1 | deg].
  - h1 node table: AllGather (node-sharded -> replicated); conv2 gathers
    h1[src] rows with per-tile indirect DMAs on the otherwise-idle GPSIMD.
  - global_add_pool: one-hot(batch) matmuls into PSUM; AllReduce of [64,16];
    MLP head replicated on every core.
"""

import sys

sys.path.insert(0, "/opt/trn_rl_repo")

import numpy as np

ml_bf16 = np.float16

import concourse.bass as bass
import concourse.bacc as bacc
import concourse.tile as tile
from concourse import mybir
from concourse import bass_utils
from concourse.bass_interp import get_hw_module

F32 = mybir.dt.float32
I32 = mybir.dt.int32

N_NODES, N_EDGES, N_GRAPHS = 50000, 400000, 64
NF, EF = 16, 8
NCORES = 8
NPC = N_NODES // NCORES          # 6250 nodes owned per core
WIN = 128                        # nodes per scatter window
NWIN = (NPC + WIN - 1) // WIN    # 49 windows per core
NPAD = NWIN * WIN                # 6272 padded nodes per core
P = 128
NT_NODE = NPAD // P              # node-major tiles per core (= NWIN)
PAD_DST = 300.0                  # sentinel: one-hot never matches
CHUNK_T_MAX = 44                 # max tiles per streamed hT chunk
BF16 = mybir.dt.float16  # 16-bit path: fp16 (better mantissa than bf16)

_cache = {}
DEBUG_TAPS = False  # add intermediate-dump outputs to the program
SKIP_COLLECTIVES = False  # replace collectives with local DMAs (1-core sim)


# --------------------------------------------------------------------------
# Host-side preparation
# --------------------------------------------------------------------------
def _host_prep(inputs):
    x = np.asarray(inputs["x"], np.float32)
    ei = np.asarray(inputs["edge_index"])
    ea = np.asarray(inputs["edge_attr"], np.float32)
    batch = np.asarray(inputs["batch"]).astype(np.int64)
    src = ei[0].astype(np.int64)
    dst = ei[1].astype(np.int64)

    gw = {k: np.asarray(inputs[k], np.float32) for k in (
        "c1_w1", "c1_b1", "c1_w2", "c1_b2", "c1_root", "c1_bias",
        "c2_w1", "c2_b1", "c2_w2", "c2_b2", "c2_root", "c2_bias",
        "fc1_w", "fc1_b", "out_w", "out_b")}

    # tiny first MLP layers (edge feature prep, host)
    h1e = np.maximum(ea @ gw["c1_w1"] + gw["c1_b1"], 0.0)   # [E, 32]
    h2e = np.maximum(ea @ gw["c2_w1"] + gw["c2_b1"], 0.0)   # [E, 32]
    xs_full = x[src]                                        # [E, 16]
    srcg_full = (src // NPC) * NPAD + (src % NPC)           # gather row ids
    deg = np.bincount(dst, minlength=N_NODES).astype(np.float32)

    # sort edges by dst; contiguous per-core slices
    order = np.argsort(dst, kind="stable")
    dst_s = dst[order]
    core_bounds = np.searchsorted(dst_s, np.arange(NCORES + 1) * NPC)

    # per (core, window) edge counts
    wcnt = np.zeros((NCORES, NWIN), np.int64)
    for k in range(NCORES):
        lo, hi = core_bounds[k], core_bounds[k + 1]
        dl = dst_s[lo:hi] - k * NPC
        wb = np.searchsorted(dl, np.arange(NWIN + 1) * WIN)
        wcnt[k] = np.diff(wb)

    tiles_w = np.maximum(np.ceil(wcnt / P).astype(np.int64).max(axis=0), 0)
    tile_start = np.concatenate([[0], np.cumsum(tiles_w)])
    T = int(tile_start[-1])

    # schedule: chunks of consecutive windows, each <= CHUNK_T_MAX-3
    # tiles; pad each chunk's tile count to a multiple of 4 (for the
    # 4-deep row-tiled We matmul layout)
    chunks = []
    w = 0
    while w < NWIN:
        w2 = w + 1
        while (w2 < NWIN
               and tile_start[w2 + 1] - tile_start[w] <= CHUNK_T_MAX - 3):
            w2 += 1
        pad = (-(tile_start[w2] - tile_start[w])) % 4
        tiles_w[w2 - 1] += pad
        tile_start = np.concatenate([[0], np.cumsum(tiles_w)])
        chunks.append((w, w2))
        w = w2
    T = int(tile_start[-1])
    sched = (T, tuple(int(t) for t in tiles_w), tuple(chunks))

    b2sum1 = gw["c1_b2"].reshape(NF, 32).sum(0)     # [32]
    b2sum2 = gw["c2_b2"].reshape(32, 16).sum(0)     # [16]

    # per-core slot-padded arrays
    per_core = []
    for k in range(NCORES):
        lo, hi = core_bounds[k], core_bounds[k + 1]
        eid = order[lo:hi]
        dl = dst_s[lo:hi] - k * NPC
        wb = np.searchsorted(dl, np.arange(NWIN + 1) * WIN)
        pos = np.arange(hi - lo)
        wof = np.searchsorted(np.arange(1, NWIN + 1) * WIN, dl, side="right")
        slot = (tile_start[wof] * P) + (pos - wb[wof])

        S = T * P
        xs = np.zeros((S, NF), np.float32)
        xs[slot] = xs_full[eid]
        hT1 = np.zeros((32, S), np.float32)
        hT1[:, slot] = h1e[eid].T
        hT2 = np.zeros((32, S), np.float32)
        hT2[:, slot] = h2e[eid].T
        dstrel = np.full(S, PAD_DST, np.float32)
        dstrel[slot] = (dl - wof * WIN).astype(np.float32)
        srcg = np.zeros(S, np.int32)
        srcg[slot] = srcg_full[eid].astype(np.int32)

        # augmented node features for the root matmul: [x | 1 | deg]
        xownT = np.zeros((NF + 2, NPAD), np.float32)
        xownT[:NF, :NPC] = x[k * NPC:(k + 1) * NPC].T
        xownT[NF, :NPC] = 1.0
        xownT[NF + 1, :NPC] = deg[k * NPC:(k + 1) * NPC]
        nconst = np.zeros((2, NPAD), np.float32)     # [1 | deg] rows for h1T
        nconst[0, :NPC] = 1.0
        nconst[1, :NPC] = deg[k * NPC:(k + 1) * NPC]
        blocal = np.full(NPAD, -1.0, np.float32)
        blocal[:NPC] = batch[k * NPC:(k + 1) * NPC].astype(np.float32)

        def stack4(hT):
            # [32, T*128] -> [128, (T//4)*128]: tile t at rows 32*(t%4)
            r = hT.reshape(32, T // 4, 4, P)
            return np.ascontiguousarray(
                r.transpose(2, 0, 1, 3).reshape(P, (T // 4) * P))

        per_core.append(dict(
            xs=np.ascontiguousarray(
                xs.reshape(T, P, NF).transpose(1, 0, 2)
                .reshape(P, T * NF).astype(np.float32)).astype(ml_bf16),
            ht1=stack4(hT1).astype(ml_bf16),
            ht2=stack4(hT2).astype(ml_bf16),
            dstrel=np.ascontiguousarray(dstrel.reshape(T, P).T),
            srcg=np.ascontiguousarray(srcg.reshape(T, P).T),
            xownT=xownT,
            nconst=nconst,
            blocal=np.ascontiguousarray(blocal.reshape(NT_NODE, P).T),
        ))

    # shared weight tensors
    def perm_oi(w2, in_c, out_c):
        # [32, out_c*in_c] in (o,i)-major order
        return np.ascontiguousarray(
            w2.reshape(32, in_c, out_c).transpose(0, 2, 1).reshape(32, -1))

    shared = dict(
        w2a=np.tile(perm_oi(gw["c1_w2"], NF, 32), (4, 1)).astype(ml_bf16),
        w2b=np.tile(perm_oi(gw["c2_w2"], 32, 16), (4, 1)).astype(ml_bf16),
        # root matmul rhs: [root; bias; b2sum]
        root1=np.ascontiguousarray(np.concatenate(
            [gw["c1_root"], gw["c1_bias"][None, :], b2sum1[None, :]], 0)),
        root2=np.ascontiguousarray(np.concatenate(
            [gw["c2_root"], gw["c2_bias"][None, :], b2sum2[None, :]], 0)),
        iota128=np.ascontiguousarray(
            np.broadcast_to(np.arange(WIN, dtype=np.float32),
                            (P, WIN))).astype(ml_bf16),
        iota64=np.ascontiguousarray(
            np.broadcast_to(np.arange(64, dtype=np.float32), (P, 64))),
        ident=np.eye(P, dtype=np.float32),
        fc1w=gw["fc1_w"], fc1b=gw["fc1_b"].reshape(32, 1),
        outw=gw["out_w"], outb=gw["out_b"].reshape(1, 1),
    )
    return sched, per_core, shared


# --------------------------------------------------------------------------
# Device program
# --------------------------------------------------------------------------
def _build_program(sched):
    T, tiles_w, chunks = sched
    tile_start = np.concatenate([[0], np.cumsum(tiles_w)]).astype(int)

    nc = bacc.Bacc("TRN2", target_bir_lowering=False, debug=False,
                   enable_asserts=False, num_devices=NCORES,
                   num_swdge_queues=4)

    def din(name, shape, dt=F32):
        return nc.dram_tensor(name, list(shape), dt, kind="ExternalInput").ap()

    xs_d = din("xs", (P, T * NF), BF16)
    ht1_d = din("ht1", (P, (T // 4) * P), BF16)
    ht2_d = din("ht2", (P, (T // 4) * P), BF16)
    dstrel_d = din("dstrel", (P, T))
    srcg_d = din("srcg", (P, T), I32)
    xownT_d = din("xownT", (NF + 2, NPAD))
    nconst_d = din("nconst", (2, NPAD))
    blocal_d = din("blocal", (P, NT_NODE))
    w2a_d = din("w2a", (P, 512), BF16)
    w2b_d = din("w2b", (P, 512), BF16)
    root1_d = din("root1", (NF + 2, 32))
    root2_d = din("root2", (34, 16))
    iota128_d = din("iota128", (P, WIN), BF16)
    iota64_d = din("iota64", (P, 64))
    ident_d = din("ident", (P, P))
    fc1w_d = din("fc1w", (NF, 32))
    fc1b_d = din("fc1b", (32, 1))
    outw_d = din("outw", (32, 1))
    outb_d = din("outb", (1, 1))
    y_d = nc.dram_tensor("y", [1, 64], F32, kind="ExternalOutput").ap()
    taps = {}
    if DEBUG_TAPS:
        for nm, shape in [("t_h1nm", (P, NT_NODE * 32)),
                          ("t_h1s0", (P, 32)), ("t_g", (64, 16))]:
            taps[nm] = nc.dram_tensor(nm, list(shape), F32,
                                      kind="ExternalOutput").ap()

    with tile.TileContext(nc) as tc:
        with (
            tc.tile_pool(name="const", bufs=1) as cp,
            tc.tile_pool(name="stream", bufs=3) as sp,
            tc.tile_pool(name="work", bufs=4) as wp,
            tc.tile_pool(name="psum", bufs=2, space="PSUM") as pp,
            tc.tile_pool(name="psum_agg", bufs=2, space="PSUM") as pagg,
            tc.tile_pool(name="dram", bufs=1, space="DRAM") as dp,
        ):
            # ---- persistent SBUF loads
            def load(dram_ap, shape, dt=F32, tag=None):
                t = cp.tile(list(shape), dt, tag=tag)
                nc.sync.dma_start(t[:], dram_ap)
                return t

            xs_s = load(xs_d, (P, T * NF), BF16, tag="xs_s")
            dstrel_s = load(dstrel_d, (P, T), tag="dstrel_s")
            srcg_s = load(srcg_d, (P, T), I32, tag="srcg_s")
            xownT_s = load(xownT_d, (NF + 2, NPAD), tag="xownT_s")
            blocal_s = load(blocal_d, (P, NT_NODE), tag="blocal_s")
            w2a_s = load(w2a_d, (P, 512), BF16, tag="w2a_s")
            w2b_s = load(w2b_d, (P, 512), BF16, tag="w2b_s")
            root1_s = load(root1_d, (NF + 2, 32), tag="root1_s")
            root2_s = load(root2_d, (34, 16), tag="root2_s")
            iota128_s = load(iota128_d, (P, WIN), BF16, tag="iota128_s")
            iota64_s = load(iota64_d, (P, 64), tag="iota64_s")
            ident_s = load(ident_d, (P, P), tag="ident_s")
            fc1w_s = load(fc1w_d, (NF, 32), tag="fc1w_s")
            fc1b_s = load(fc1b_d, (32, 1), tag="fc1b_s")
            outw_s = load(outw_d, (32, 1), tag="outw_s")
            outb_s = load(outb_d, (1, 1), tag="outb_s")

            # node tables (node-major) + feature-major h1 for conv2 root
            h1nm = cp.tile([P, NT_NODE * 32], F32, tag="h1nm")
            h2nm = cp.tile([P, NT_NODE * 16], F32, tag="h2nm")
            h1T = cp.tile([34, NPAD], F32, tag="h1T")
            nc.sync.dma_start(h1T[32:34, :], nconst_d)

            # DRAM internals for collectives
            ag_in = dp.tile([NPAD, 32], BF16, tag="ag_in")
            ag_out = dp.tile([NCORES * NPAD, 32], BF16, tag="ag_out",
                             addr_space="Shared")
            ar_in = dp.tile([64, 16], F32, tag="ar_in")
            ar_out = dp.tile([64, 16], F32, tag="ar_out",
                             addr_space="Shared")

            # ------------------------------------------------------------
            def conv_layer(ht_d, w2_s, in_c, out_c, root_lhsT, root_rhs,
                           hout_nm, src_view):
                """One NNConv layer; writes node-major relu output into
                hout_nm ([P, NT_NODE*out_c], window w at cols [w*out_c:])."""
                for (wlo, whi) in chunks:
                    clo, chi = tile_start[wlo], tile_start[whi]
                    ct = chi - clo
                    if ct > 0:
                        ht_c = sp.tile([P, (CHUNK_T_MAX // 4) * P], BF16,
                                       tag="ht_c")
                        nc.sync.dma_start(
                            ht_c[:, :(ct // 4) * P],
                            ht_d[:, (clo // 4) * P:(chi // 4) * P])
                    for w in range(wlo, whi):
                        nw = int(tiles_w[w])
                        root_ps = pp.tile([P, out_c], F32, tag="aux")
                        nc.tensor.matmul(
                            root_ps[:], lhsT=root_lhsT[:, w * WIN:(w + 1) * WIN],
                            rhs=root_rhs, start=True, stop=True)
                        if nw > 0:
                            unred = pagg.tile([P, 512], F32, tag="unred")
                        for ti in range(nw):
                            t = int(tile_start[w]) + ti
                            tl = t - clo
                            # We = hT.T @ w2 -> [128e, 512] PSUM
                            # (4x row-tiled: tile t uses PE rows 32*(t%4))
                            g4 = t % 4
                            we = pp.tile([P, 512], F32, tag="we", bufs=4)
                            nc.tensor.matmul(
                                we[:],
                                lhsT=ht_c[32 * g4:32 * (g4 + 1),
                                          (tl // 4) * P:(tl // 4 + 1) * P],
                                rhs=w2_s[32 * g4:32 * (g4 + 1), :],
                                start=True, stop=True,
                                tile_position=(32 * g4, 0))
                            # evacuate+cast We -> SBUF bf16 (on ACT)
                            we_sb = wp.tile([P, 512], BF16, tag="we_sb")
                            nc.scalar.activation(
                                out=we_sb[:], in_=we[:],
                                func=mybir.ActivationFunctionType.Copy)
                            # one-hot A [128, WIN] bf16
                            A = wp.tile([P, WIN], BF16, tag="A")
                            nc.vector.tensor_scalar(
                                out=A[:], in0=iota128_s[:],
                                scalar1=dstrel_s[:, t:t + 1], scalar2=None,
                                op0=mybir.AluOpType.is_equal)
                            # tmp = xs_bcast * We   [128, out_c, in_c] bf16
                            tmp = wp.tile([P, out_c, in_c], BF16, tag="tmp")
                            we3 = we_sb[:].rearrange(
                                "p (o i) -> p o i", o=out_c, i=in_c)
                            src_b = src_view(t, tl)[:, None, :] \
                                .broadcast_to([P, out_c, in_c])
                            nc.vector.tensor_tensor(
                                out=tmp[:], in0=we3, in1=src_b,
                                op=mybir.AluOpType.mult)
                            # scatter UNREDUCED: unred += A.T @ tmp
                            nc.tensor.matmul(
                                unred[:], lhsT=A[:],
                                rhs=tmp[:].rearrange("p o i -> p (o i)"),
                                start=(ti == 0), stop=(ti == nw - 1))
                        # combine: h = relu(reduce_i(unred) + root)
                        ocol = slice(w * out_c, (w + 1) * out_c)
                        if nw > 0:
                            r = wp.tile([P, out_c], F32, tag="r")
                            nc.vector.tensor_reduce(
                                out=r[:],
                                in_=unred[:].rearrange(
                                    "p (o i) -> p o i", o=out_c, i=in_c),
                                axis=mybir.AxisListType.X,
                                op=mybir.AluOpType.add)
                            s = wp.tile([P, out_c], F32, tag="s")
                            nc.vector.tensor_tensor(
                                out=s[:], in0=r[:], in1=root_ps[:],
                                op=mybir.AluOpType.add)
                        else:
                            s = wp.tile([P, out_c], F32, tag="s")
                            nc.vector.tensor_copy(s[:], root_ps[:])
                        nc.scalar.activation(
                            out=hout_nm[:, ocol], in_=s[:],
                            func=mybir.ActivationFunctionType.Relu)

            # ---- conv1
            conv_layer(ht1_d, w2a_s, NF, 32, xownT_s[:], root1_s[:], h1nm,
                       lambda t, tl: xs_s[:, t * NF:(t + 1) * NF])

            # ---- ship h1 (node-major) to AllGather; build feature-major h1T
            nc.gpsimd.dma_start(
                ag_in[:].rearrange("(t p) f -> p t f", p=P),
                h1nm[:].rearrange("p (t f) -> p t f", f=32))
            if SKIP_COLLECTIVES:
                nc.sync.dma_start(ag_out[:NPAD, :], ag_in[:])
            else:
                nc.gpsimd.collective_compute(
                    "AllGather", mybir.AluOpType.bypass,
                    replica_groups=[list(range(NCORES))],
                    ins=[ag_in[:].opt()], outs=[ag_out[:].opt()])
            for nt in range(NT_NODE):
                tp = pp.tile([32, P], F32, tag="aux")
                nc.tensor.transpose(
                    tp[:], in_=h1nm[:, nt * 32:(nt + 1) * 32],
                    identity=ident_s[:, :])
                nc.scalar.activation(
                    out=h1T[:32, nt * P:(nt + 1) * P], in_=tp[:],
                    func=mybir.ActivationFunctionType.Copy)
            if DEBUG_TAPS:
                nc.sync.dma_start(taps["t_h1nm"], h1nm[:])

            # ---- conv2: gather h1[src] per tile (canonical [128,1]-offset
            # indirect DMA on GPSIMD; overlaps PE/DVE compute)
            def h1s_tile(t, tl):
                g = wp.tile([P, 32], BF16, tag="h1s_t", bufs=16)
                inst = nc.gpsimd.indirect_dma_start(
                    out=g[:], out_offset=None, in_=ag_out[:],
                    in_offset=bass.IndirectOffsetOnAxis(
                        ap=srcg_s[:, t:t + 1], axis=0),
                )
                if t % 4:
                    inst.queue = "qPoolDynamic%d" % (t % 4)
                if DEBUG_TAPS and t == 0:
                    nc.sync.dma_start(taps["t_h1s0"], g[:])
                return g[:]

            conv_layer(ht2_d, w2b_s, 32, 16, h1T[:], root2_s[:], h2nm,
                       h1s_tile)

            # ---- global_add_pool: one-hot(batch) matmuls, node-major h2
            g_ps = pp.tile([64, 16], F32, tag="aux")
            for nt in range(NT_NODE):
                B = wp.tile([P, 64], F32, tag="B")
                nc.vector.tensor_scalar(
                    out=B[:], in0=iota64_s[:],
                    scalar1=blocal_s[:, nt:nt + 1], scalar2=None,
                    op0=mybir.AluOpType.is_equal)
                nc.tensor.matmul(
                    g_ps[:], lhsT=B[:], rhs=h2nm[:, nt * 16:(nt + 1) * 16],
                    start=(nt == 0), stop=(nt == NT_NODE - 1))
            g_s = wp.tile([64, 16], F32, tag="g_s")
            nc.vector.tensor_copy(g_s[:], g_ps[:])
            if DEBUG_TAPS:
                nc.sync.dma_start(taps["t_g"], g_s[:])
            nc.sync.dma_start(ar_in[:], g_s[:])
            if SKIP_COLLECTIVES:
                nc.sync.dma_start(ar_out[:], ar_in[:])
            else:
                nc.gpsimd.collective_compute(
                    "AllReduce", mybir.AluOpType.add,
                    replica_groups=[list(range(NCORES))],
                    ins=[ar_in[:].opt()], outs=[ar_out[:].opt()])
            g_r = wp.tile([64, 16], F32, tag="g_r")
            nc.sync.dma_start(g_r[:], ar_out[:])

            # ---- head: y = relu(g@fc1+b)@out_w + out_b
            gT_ps = pp.tile([16, 64], F32, tag="aux")
            nc.tensor.transpose(gT_ps[:], in_=g_r[:], identity=ident_s[:64, :64])
            gT_s = wp.tile([16, 64], F32, tag="gT_s")
            nc.vector.tensor_copy(gT_s[:], gT_ps[:])
            o1 = pp.tile([32, 64], F32, tag="aux")
            nc.tensor.matmul(o1[:], lhsT=fc1w_s[:], rhs=gT_s[:],
                             start=True, stop=True)
            r1 = wp.tile([32, 64], F32, tag="r1")
            nc.scalar.activation(out=r1[:], in_=o1[:],
                                 func=mybir.ActivationFunctionType.Relu,
                                 bias=fc1b_s[:])
            o2 = pp.tile([1, 64], F32, tag="aux")
            nc.tensor.matmul(o2[:], lhsT=outw_s[:], rhs=r1[:],
                             start=True, stop=True)
            ys = wp.tile([1, 64], F32, tag="ys")
            nc.vector.tensor_scalar(out=ys[:], in0=o2[:],
                                    scalar1=outb_s[:], scalar2=None,
                                    op0=mybir.AluOpType.add)
            nc.sync.dma_start(y_d, ys[:])

    nc.compile()
    nc.m = get_hw_module(nc.m)
    return nc


# --------------------------------------------------------------------------
def kernel(**inputs):
    sched, per_core, shared = _host_prep(inputs)
    key = sched
    if key not in _cache:
        _cache[key] = _build_program(sched)
    nc = _cache[key]

    in_maps = []
    for k in range(NCORES):
        m = dict(shared)
        m.update(per_core[k])
        m = {n: np.ascontiguousarray(v) for n, v in m.items()}
        in_maps.append(m)

    res = bass_utils.run_bass_kernel_spmd(nc, in_maps,
                                          core_ids=list(range(NCORES)))
    y = np.asarray(res.results[0]["y"], np.float32).reshape(64, 1)
    return y

